# revision 26
# baseline (speedup 1.0000x reference)
"""Trainium2 Bass kernel for nn_BackboneEncoderGNN (B=1, N=768, DN=256, DE=128).

Strategy (8 NeuronCores, residue-axis sharding, 96 rows each):
  * All edge features are produced through bilinear decompositions so the
    TensorEngine does the heavy lifting:
      - 4x4 atom log-distances: ss = |Xi|^2 + |Xj|^2 - 2 Xi.Xj via a K=20
        "pre-matmul" (host-built augmented factors), then ScalarE computes
        ln(sqrt(max(ss,TAU)+1e-8) + 1e-2) whole-tile.
      - trel (frame-projected translation) + |Ca_i - Ca_j|^2 via a K=5
        pre-matmul; unit-normalisation reuses the Ca-Ca distance (rotation
        invariance) with VectorE reciprocal.
      - relative-rotation block is linear in R_j, so it is folded into
        per-i transformed weights W'_i = einsum(R_i, W_R) on the host.
  * One fp32 matmul per (4-row igroup, 128-j chunk): lhsT = assembled
    [94, 128] feature tile, rhs = host-built block-diagonal [94, 512]
    weight tile selecting each i; PSUM [128 j, 4i x 128 de].
  * One fused DVE op per tile: out = psum * mask_j + aux_masked.
  * i==j diagonal: catastrophic cancellation makes PE-computed ss/trel
    garbage there; the TAU clamp pins the device value to a deterministic
    constant and a host-side correction folded into the aux input restores
    the exact reference diagonal.
Host prepares the small O(N) factor matrices; device moves/computes the
O(N^2 * DE) edge block (~75 MB per core: memory-roofline bound).
"""
import numpy as np
from contextlib import ExitStack

B, N = 1, 768
DN, DE = 256, 128
EPS = 1e-8
NCORES = 8
NI = N // NCORES        # 96 rows per core
IG = 4                  # i's per final-matmul group
NG = NI // IG           # 24 igroups per core
TAU = 0.02              # ss clamp: PE cancellation noise (<=8e-4) << TAU
KALL = 94               # final matmul contraction rows


# ---------------------------------------------------------------------------
# host math helpers (mirror reference.py formulas)
# ---------------------------------------------------------------------------

def _frames(Na, Ca, Cc):
    def unit(v):
        return v / np.sqrt(np.sum(v * v, -1, keepdims=True) + EPS)
    u = unit(Cc - Ca)
    v = unit(Na - Ca)
    e1 = u
    e2 = unit(v - np.sum(v * e1, -1, keepdims=True) * e1)
    e3 = np.cross(e1, e2)
    return np.stack([e1, e2, e3], axis=-2)  # (N,3,3) rows = basis


def _node_features(X):
    def norm(v):
        return np.sqrt(np.sum(v * v, -1) + EPS)
    def unit(v):
        return v / norm(v)[..., None]
    Na, Ca, Cc, Ox = X[:, 0], X[:, 1], X[:, 2], X[:, 3]
    d1 = np.log(norm(Ca - Na) + 1e-3)
    d2 = np.log(norm(Cc - Ca) + 1e-3)
    d3 = np.log(norm(Ox - Cc) + 1e-3)
    a1 = np.sum(unit(Na - Ca) * unit(Cc - Ca), -1)
    a2 = np.sum(unit(Ca - Cc) * unit(Ox - Cc), -1)
    b0, b1, b2 = Ca - Na, Cc - Ca, Ox - Cc
    n1 = np.cross(b0, b1)
    n2 = np.cross(b1, b2)
    m1 = np.cross(n1, unit(b1))
    x = np.sum(n1 * n2, -1)
    y = np.sum(m1 * n2, -1)
    r = np.sqrt(x * x + y * y + EPS)
    return np.stack([d1, d2, d3, a1, a2, y / r, x / r], -1)  # (N,7)


def _prep(X, C, node_h_aux, edge_h_aux, W_node, b_node, W_edge, b_edge):
    """Build per-core device input dicts + host-only outputs."""
    X = np.asarray(X, np.float64)[0]              # (N,4,3)
    C = np.asarray(C)[0]
    aux = np.asarray(edge_h_aux, np.float32)[0]   # (N,N,DE)
    naux = np.asarray(node_h_aux, np.float32)[0]
    W_node = np.asarray(W_node, np.float64)
    b_node = np.asarray(b_node, np.float64)
    W_edge = np.asarray(W_edge, np.float64)
    b_edge = np.asarray(b_edge, np.float64)

    m = (C > 0).astype(np.float32)
    mask_ij = m[:, None] * m[None, :]
    all_valid = bool(np.all(m == 1.0))

    R = _frames(X[:, 0], X[:, 1], X[:, 2])
    Ca = X[:, 1]
    Xsq = np.sum(X * X, -1)                       # (N,4)
    d_i = np.einsum('ipc,ic->ip', R, Ca)
    Casq = Xsq[:, 1]

    W_D = W_edge[0:16]
    W_R = W_edge[16:25].reshape(3, 3, DE)
    W_T = W_edge[25:28]
    w_sc = W_edge[28]
    w_sep = W_edge[29]
    Wp = np.einsum('ipc,pqd->iqcd', R, W_R).reshape(N, 9, DE)

    rhs_preD = np.zeros((20, N), np.float64)
    for a2 in range(4):
        rhs_preD[a2 * 5 + 0: a2 * 5 + 3] = -2.0 * X[:, a2, :].T
        rhs_preD[a2 * 5 + 3] = 1.0
        rhs_preD[a2 * 5 + 4] = Xsq[:, a2]
    rhs_preT = np.zeros((5, N), np.float64)
    rhs_preT[0:3] = Ca.T
    rhs_preT[3] = 1.0
    rhs_preT[4] = Casq
    rj_ones = np.zeros((10, N), np.float64)
    rj_ones[0:9] = R.reshape(N, 9).T
    rj_ones[9] = 1.0

    seq = np.arange(N)
    sep_full = np.clip(seq[None, :] - seq[:, None], -32, 32).astype(np.float64) / 32.0
    sc_full = (C[:, None] == C[None, :]).astype(np.float64)

    # --- clamp corrections -------------------------------------------------
    # The device computes ss via the PE Gram trick and clamps max(ss, TAU).
    # For every (i,j,f) with true ss below TAU the device value is the
    # deterministic K_tau; add (D_true - K_tau) @ W_D into aux on the host.
    # Same for the trel_u normaliser when |Ca_i - Ca_j|^2 < TAU.
    K_tau = np.log(np.sqrt(TAU + EPS) + 1e-2)
    ss_all = np.sum(
        (X[:, None, :, None, :] - X[None, :, None, :, :]) ** 2, -1
    ).reshape(N, N, 16)                                  # (i, j, a1*4+a2)
    clamped = ss_all < TAU                               # includes diagonal
    corr_map = {}                                        # (i,j) -> (DE,)
    for i, j in zip(*np.nonzero(clamped.any(-1))):
        f = np.nonzero(clamped[i, j])[0]
        d_true = np.log(np.sqrt(ss_all[i, j, f] + EPS) + 1e-2)
        corr_map[(i, j)] = (d_true - K_tau) @ W_D[f]
    css = np.sum((Ca[:, None, :] - Ca[None, :, :]) ** 2, -1)
    for i, j in zip(*np.nonzero(css < TAU)):
        trel_r = R[i] @ (Ca[j] - Ca[i])                  # (3,)
        tu_true = trel_r / np.sqrt(np.sum(trel_r ** 2) + EPS)
        tu_dev = trel_r / np.sqrt(TAU + EPS)
        dv = (tu_true - tu_dev) @ W_T
        corr_map[(i, j)] = corr_map.get((i, j), 0.0) + dv

    node_feat = _node_features(X)

    cores = []
    ar4 = np.arange(IG)
    for c in range(NCORES):
        rows = slice(c * NI, (c + 1) * NI)
        idx = np.arange(c * NI, (c + 1) * NI)

        aux_c = np.empty((NI, N, DE), np.float32)
        np.copyto(aux_c, aux[rows])
        if not all_valid:
            aux_c *= mask_ij[rows][:, :, None]
        for (ci, cj), dv in corr_map.items():
            if c * NI <= ci < (c + 1) * NI:
                aux_c[ci - c * NI, cj, :] += \
                    (m[ci] * m[cj] * dv).astype(np.float32)

        gidx = idx.reshape(NG, IG)                     # (NG, IG) global i

        # lhsT_pre (20, NG*128): ONE pre-matmul per (igroup, half).
        # M-col layout per igroup:
        #   0:64    D features, f-major (f*IG+s), f = a1*4+a2
        #   64:76   ssCaCa tripled (64 + p*IG + s) -> rnorm source
        #   76:96   zero pad (alignment)
        #   96:108  trel (96 + p*IG + s), via the a2=1 rows of rhs_preD:
        #           (-R_i[p]/2)·(-2Ca_j) + (-d_i[p])·1 = R_i[p]·Ca_j - d_i[p]
        #   108:128 zero pad
        lp = np.zeros((20, NG * 128), np.float64)
        Xi = X[gidx]                                   # (NG, IG, 4, 3)
        Xsqi = Xsq[gidx]                               # (NG, IG, 4)
        for f in range(16):
            a1_, a2_ = f // 4, f % 4
            cols = (np.arange(NG)[:, None] * 128 + f * IG + ar4[None, :])
            lp[a2_ * 5 + 0: a2_ * 5 + 3, cols] = \
                Xi[:, :, a1_, :].transpose(2, 0, 1)
            lp[a2_ * 5 + 3, cols] = Xsqi[:, :, a1_]
            lp[a2_ * 5 + 4, cols] = 1.0
        for g in range(NG):
            for s in range(IG):
                i = gidx[g, s]
                for p in range(3):
                    col = g * 128 + 64 + p * IG + s
                    lp[5:8, col] = Ca[i]
                    lp[8, col] = Casq[i]
                    lp[9, col] = 1.0
                    col = g * 128 + 96 + p * IG + s
                    lp[5:8, col] = -R[i, p, :] / 2.0
                    lp[8, col] = -d_i[i, p]

        # scrj (NG, 18, N): rows 0:8 = sc/sep, 8:17 = R_j^T, 17 = ones;
        # DMA'd in one shot into L[76:94]
        ss_rows = np.zeros((NG, 18, N), np.float64)
        ss_rows[:, 8:17, :] = rj_ones[0:9]
        ss_rows[:, 17, :] = 1.0
        for g in range(NG):
            for s in range(IG):
                i = gidx[g, s]
                ss_rows[g, s] = sc_full[i]
                ss_rows[g, IG + s] = sep_full[i]

        # rhs_all (NG, 94, IG*DE)
        ra = np.zeros((NG, KALL, IG * DE), np.float64)
        for g in range(NG):
            for s in range(IG):
                i = gidx[g, s]
                cs = slice(s * DE, (s + 1) * DE)
                mi = m[i]
                for f in range(16):
                    ra[g, f * IG + s, cs] = mi * W_D[f]
                for p in range(3):
                    ra[g, 64 + p * IG + s, cs] = mi * W_T[p]
                ra[g, 76 + s, cs] = mi * w_sc
                ra[g, 80 + s, cs] = mi * w_sep
                ra[g, 84:93, cs] = mi * Wp[i]
                ra[g, 93, cs] = mi * b_edge

        nodeT = node_feat[rows].T
        node_aux = m[rows, None] * (naux[rows].astype(np.float64) + b_node)

        cores.append(dict(
            aux=aux_c,
            rhs_all=np.ascontiguousarray(ra, dtype=np.float32),
            lhsT_preD=lp.astype(np.float32),
            rhs_preD=rhs_preD.astype(np.float32),
            scrj=np.ascontiguousarray(ss_rows, dtype=np.float32),
            maskj=np.ascontiguousarray(m.reshape(6, 128).T),
            nodeT=np.ascontiguousarray(nodeT, dtype=np.float32),
            w_node=W_node.astype(np.float32),
            node_aux=node_aux.astype(np.float32),
            maski=m[rows].reshape(NI, 1).copy(),
        ))

    host_out = dict(
        mask_i=m.reshape(1, N).copy(),
        mask_ij=np.ascontiguousarray(mask_ij.reshape(1, N, N)),
        edge_idx=np.ascontiguousarray(
            np.broadcast_to(np.arange(N, dtype=np.int32)[None, None, :],
                            (1, N, N))),
    )
    return cores, host_out


# ---------------------------------------------------------------------------
# device program
# ---------------------------------------------------------------------------

_NC_CACHE = None
_LAST_RESULTS = None     # stashed BassKernelResults for test harness inspection


def _build_bass(repeat=1):
    import concourse.bass as bass
    import concourse.bacc as bacc
    import concourse.tile as tile
    from concourse import mybir

    AF = mybir.ActivationFunctionType
    ALU = mybir.AluOpType
    f32 = mybir.dt.float32
    f32r = mybir.dt.float32r

    nc = bacc.Bacc("TRN2", debug=False, target_bir_lowering=False)
    d_aux = nc.dram_tensor("aux", (NI, N, DE), f32, kind="ExternalInput")
    d_ra = nc.dram_tensor("rhs_all", (NG, KALL, IG * DE), f32r, kind="ExternalInput")
    d_lp = nc.dram_tensor("lhsT_preD", (20, NG * 128), f32, kind="ExternalInput")
    d_rpD = nc.dram_tensor("rhs_preD", (20, N), f32, kind="ExternalInput")
    d_scrj = nc.dram_tensor("scrj", (NG, 18, N), f32r, kind="ExternalInput")
    d_maskj = nc.dram_tensor("maskj", (128, 6), f32, kind="ExternalInput")
    d_nodeT = nc.dram_tensor("nodeT", (7, NI), f32, kind="ExternalInput")
    d_wn = nc.dram_tensor("w_node", (7, DN), f32, kind="ExternalInput")
    d_naux = nc.dram_tensor("node_aux", (NI, DN), f32, kind="ExternalInput")
    d_maski = nc.dram_tensor("maski", (NI, 1), f32, kind="ExternalInput")
    d_edge = nc.dram_tensor("edge", (NI, N, DE), f32, kind="ExternalOutput")
    d_node = nc.dram_tensor("node", (NI, DN), f32, kind="ExternalOutput")

    with tile.TileContext(nc) as tc, ExitStack() as ctx:
        sp = ctx.enter_context(tc.tile_pool(name="statics", bufs=1))
        lp_ = sp.tile([20, NG * 128], f32)
        rpD_ = sp.tile([20, N], f32)
        maskj_ = sp.tile([128, 6], f32)
        cbias = sp.tile([128, 1], f32)
        nc.gpsimd.memset(cbias[:], 1e-2)
        nc.sync.dma_start(lp_[:], d_lp[:])
        nc.sync.dma_start(rpD_[:], d_rpD[:])
        nc.sync.dma_start(maskj_[:], d_maskj[:])

        lpool = ctx.enter_context(tc.tile_pool(name="L", bufs=3))
        rapool = ctx.enter_context(tc.tile_pool(name="ra", bufs=3))
        ppre = ctx.enter_context(
            tc.tile_pool(name="ppre", bufs=2, space=bass.MemorySpace.PSUM))
        pout = ctx.enter_context(
            tc.tile_pool(name="pout", bufs=5, space=bass.MemorySpace.PSUM))
        pnode = ctx.enter_context(
            tc.tile_pool(name="pnode", bufs=1, space=bass.MemorySpace.PSUM))
        tmp = ctx.enter_context(tc.tile_pool(name="tmp", bufs=3))
        auxp = ctx.enter_context(tc.tile_pool(name="auxp", bufs=3))
        outp = ctx.enter_context(tc.tile_pool(name="outp", bufs=3))

        for _rep in range(repeat):
          for g in range(NG):
            L = lpool.tile([KALL, N], f32r)
            nc.scalar.dma_start(L[76:KALL, :], d_scrj[g])
            ra_ = rapool.tile([KALL, IG * DE], f32r)
            nc.sync.dma_start(ra_[:], d_ra[g])

            for jh in range(2):
                js = slice(jh * 384, (jh + 1) * 384)
                pP = ppre.tile([128, 384], f32)
                nc.tensor.matmul(pP[:], lp_[:, g * 128:(g + 1) * 128],
                                 rpD_[:, js], start=True, stop=True)
                ssc = tmp.tile([76, 384], f32, tag="ssc")
                nc.vector.tensor_scalar(ssc[:], pP[0:76, :], TAU, EPS,
                                        ALU.max, ALU.add)
                nrm = tmp.tile([76, 384], f32, tag="nrm")
                nc.scalar.activation(nrm[:], ssc[:], AF.Sqrt)
                nc.scalar.activation(L[0:64, js], nrm[0:64, :], AF.Ln,
                                     bias=cbias[0:64, :])
                rn = tmp.tile([12, 384], f32, tag="rn")
                nc.vector.reciprocal(rn[:], nrm[64:76, :])
                nc.vector.tensor_tensor(L[64:76, js], pP[96:108, :], rn[:],
                                        ALU.mult)

            at = auxp.tile([128, IG, 6, DE], f32)
            for s in range(IG):
                nc.gpsimd.dma_start(
                    at[:, s, :, :],
                    d_aux[g * IG + s].rearrange("(c p) d -> p c d", p=128))
            ot = outp.tile([128, IG, 6, DE], f32)
            for jc in range(6):
                js = slice(jc * 128, (jc + 1) * 128)
                po = pout.tile([128, IG, DE], f32)
                nc.tensor.matmul(po[:], L[:, js], ra_[:],
                                 start=True, stop=True)
                nc.vector.scalar_tensor_tensor(
                    ot[:, :, jc, :], po[:], maskj_[:, jc:jc + 1],
                    at[:, :, jc, :], ALU.mult, ALU.add)
            for s in range(IG):
                nc.sync.dma_start(
                    d_edge[g * IG + s].rearrange("(c p) d -> p c d", p=128),
                    ot[:, s, :, :])

        # node path
        nt = sp.tile([7, NI], f32)
        wn = sp.tile([7, DN], f32)
        na = sp.tile([NI, DN], f32)
        mi_ = sp.tile([NI, 1], f32)
        nc.sync.dma_start(nt[:], d_nodeT[:])
        nc.sync.dma_start(wn[:], d_wn[:])
        nc.sync.dma_start(na[:], d_naux[:])
        nc.sync.dma_start(mi_[:], d_maski[:])
        pn = pnode.tile([NI, DN], f32)
        nc.tensor.matmul(pn[:], nt[:], wn[:], start=True, stop=True)
        no = sp.tile([NI, DN], f32, tag="nodeout")
        nc.vector.scalar_tensor_tensor(no[:], pn[:], mi_[:], na[:],
                                       ALU.mult, ALU.add)
        nc.sync.dma_start(d_node[:], no[:])

    nc.compile()
    return nc


def _get_nc():
    global _NC_CACHE
    if _NC_CACHE is None:
        _NC_CACHE = _build_bass()
    return _NC_CACHE


def kernel(X, C, node_h_aux, edge_h_aux, W_node, b_node, W_edge, b_edge):
    global _LAST_RESULTS
    from concourse import bass_utils

    cores, host_out = _prep(X, C, node_h_aux, edge_h_aux,
                            W_node, b_node, W_edge, b_edge)
    nc = _get_nc()
    res = bass_utils.run_bass_kernel_spmd(
        nc, cores, core_ids=list(range(NCORES)))
    _LAST_RESULTS = res

    node_h = np.concatenate(
        [res.results[c]["node"] for c in range(NCORES)], 0)[None]
    edge_h = np.concatenate(
        [res.results[c]["edge"] for c in range(NCORES)], 0)[None]
    return (node_h.astype(np.float32),
            edge_h.astype(np.float32),
            host_out["edge_idx"],
            host_out["mask_i"].astype(np.float32),
            host_out["mask_ij"].astype(np.float32))


# revision 27
# speedup vs baseline: 1.0087x; 1.0087x over previous
"""Trainium2 Bass kernel for nn_BackboneEncoderGNN (B=1, N=768, DN=256, DE=128).

Strategy (8 NeuronCores, residue-axis sharding, 96 rows each):
  * All edge features are produced through bilinear decompositions so the
    TensorEngine does the heavy lifting:
      - 4x4 atom log-distances: ss = |Xi|^2 + |Xj|^2 - 2 Xi.Xj via a K=20
        "pre-matmul" (host-built augmented factors), then ScalarE computes
        ln(sqrt(max(ss,TAU)+1e-8) + 1e-2) whole-tile.
      - trel (frame-projected translation) + |Ca_i - Ca_j|^2 via a K=5
        pre-matmul; unit-normalisation reuses the Ca-Ca distance (rotation
        invariance) with VectorE reciprocal.
      - relative-rotation block is linear in R_j, so it is folded into
        per-i transformed weights W'_i = einsum(R_i, W_R) on the host.
  * One fp32 matmul per (4-row igroup, 128-j chunk): lhsT = assembled
    [94, 128] feature tile, rhs = host-built block-diagonal [94, 512]
    weight tile selecting each i; PSUM [128 j, 4i x 128 de].
  * One fused DVE op per tile: out = psum * mask_j + aux_masked.
  * i==j diagonal: catastrophic cancellation makes PE-computed ss/trel
    garbage there; the TAU clamp pins the device value to a deterministic
    constant and a host-side correction folded into the aux input restores
    the exact reference diagonal.
Host prepares the small O(N) factor matrices; device moves/computes the
O(N^2 * DE) edge block (~75 MB per core: memory-roofline bound).
"""
import numpy as np
from contextlib import ExitStack

B, N = 1, 768
DN, DE = 256, 128
EPS = 1e-8
NCORES = 8
NI = N // NCORES        # 96 rows per core
IG = 4                  # i's per final-matmul group
NG = NI // IG           # 24 igroups per core
TAU = 0.02              # ss clamp: PE cancellation noise (<=8e-4) << TAU
KALL = 94               # final matmul contraction rows


# ---------------------------------------------------------------------------
# host math helpers (mirror reference.py formulas)
# ---------------------------------------------------------------------------

def _frames(Na, Ca, Cc):
    def unit(v):
        return v / np.sqrt(np.sum(v * v, -1, keepdims=True) + EPS)
    u = unit(Cc - Ca)
    v = unit(Na - Ca)
    e1 = u
    e2 = unit(v - np.sum(v * e1, -1, keepdims=True) * e1)
    e3 = np.cross(e1, e2)
    return np.stack([e1, e2, e3], axis=-2)  # (N,3,3) rows = basis


def _node_features(X):
    def norm(v):
        return np.sqrt(np.sum(v * v, -1) + EPS)
    def unit(v):
        return v / norm(v)[..., None]
    Na, Ca, Cc, Ox = X[:, 0], X[:, 1], X[:, 2], X[:, 3]
    d1 = np.log(norm(Ca - Na) + 1e-3)
    d2 = np.log(norm(Cc - Ca) + 1e-3)
    d3 = np.log(norm(Ox - Cc) + 1e-3)
    a1 = np.sum(unit(Na - Ca) * unit(Cc - Ca), -1)
    a2 = np.sum(unit(Ca - Cc) * unit(Ox - Cc), -1)
    b0, b1, b2 = Ca - Na, Cc - Ca, Ox - Cc
    n1 = np.cross(b0, b1)
    n2 = np.cross(b1, b2)
    m1 = np.cross(n1, unit(b1))
    x = np.sum(n1 * n2, -1)
    y = np.sum(m1 * n2, -1)
    r = np.sqrt(x * x + y * y + EPS)
    return np.stack([d1, d2, d3, a1, a2, y / r, x / r], -1)  # (N,7)


def _prep(X, C, node_h_aux, edge_h_aux, W_node, b_node, W_edge, b_edge):
    """Build per-core device input dicts + host-only outputs."""
    X = np.asarray(X, np.float64)[0]              # (N,4,3)
    C = np.asarray(C)[0]
    aux = np.asarray(edge_h_aux, np.float32)[0]   # (N,N,DE)
    naux = np.asarray(node_h_aux, np.float32)[0]
    W_node = np.asarray(W_node, np.float64)
    b_node = np.asarray(b_node, np.float64)
    W_edge = np.asarray(W_edge, np.float64)
    b_edge = np.asarray(b_edge, np.float64)

    m = (C > 0).astype(np.float32)
    mask_ij = m[:, None] * m[None, :]
    all_valid = bool(np.all(m == 1.0))

    R = _frames(X[:, 0], X[:, 1], X[:, 2])
    Ca = X[:, 1]
    Xsq = np.sum(X * X, -1)                       # (N,4)
    d_i = np.einsum('ipc,ic->ip', R, Ca)
    Casq = Xsq[:, 1]

    W_D = W_edge[0:16]
    W_R = W_edge[16:25].reshape(3, 3, DE)
    W_T = W_edge[25:28]
    w_sc = W_edge[28]
    w_sep = W_edge[29]
    Wp = np.einsum('ipc,pqd->iqcd', R, W_R).reshape(N, 9, DE)

    rhs_preD = np.zeros((20, N), np.float64)
    for a2 in range(4):
        rhs_preD[a2 * 5 + 0: a2 * 5 + 3] = -2.0 * X[:, a2, :].T
        rhs_preD[a2 * 5 + 3] = 1.0
        rhs_preD[a2 * 5 + 4] = Xsq[:, a2]
    rhs_preT = np.zeros((5, N), np.float64)
    rhs_preT[0:3] = Ca.T
    rhs_preT[3] = 1.0
    rhs_preT[4] = Casq
    rj_ones = np.zeros((10, N), np.float64)
    rj_ones[0:9] = R.reshape(N, 9).T
    rj_ones[9] = 1.0

    seq = np.arange(N)
    sep_full = np.clip(seq[None, :] - seq[:, None], -32, 32).astype(np.float64) / 32.0
    sc_full = (C[:, None] == C[None, :]).astype(np.float64)

    # --- clamp corrections -------------------------------------------------
    # The device computes ss via the PE Gram trick and clamps max(ss, TAU).
    # For every (i,j,f) with true ss below TAU the device value is the
    # deterministic K_tau; add (D_true - K_tau) @ W_D into aux on the host.
    # Same for the trel_u normaliser when |Ca_i - Ca_j|^2 < TAU.
    K_tau = np.log(np.sqrt(TAU + EPS) + 1e-2)
    ss_all = np.sum(
        (X[:, None, :, None, :] - X[None, :, None, :, :]) ** 2, -1
    ).reshape(N, N, 16)                                  # (i, j, a1*4+a2)
    clamped = ss_all < TAU                               # includes diagonal
    corr_map = {}                                        # (i,j) -> (DE,)
    for i, j in zip(*np.nonzero(clamped.any(-1))):
        f = np.nonzero(clamped[i, j])[0]
        d_true = np.log(np.sqrt(ss_all[i, j, f] + EPS) + 1e-2)
        corr_map[(i, j)] = (d_true - K_tau) @ W_D[f]
    css = np.sum((Ca[:, None, :] - Ca[None, :, :]) ** 2, -1)
    for i, j in zip(*np.nonzero(css < TAU)):
        trel_r = R[i] @ (Ca[j] - Ca[i])                  # (3,)
        tu_true = trel_r / np.sqrt(np.sum(trel_r ** 2) + EPS)
        tu_dev = trel_r / np.sqrt(TAU + EPS)
        dv = (tu_true - tu_dev) @ W_T
        corr_map[(i, j)] = corr_map.get((i, j), 0.0) + dv

    node_feat = _node_features(X)

    cores = []
    ar4 = np.arange(IG)
    for c in range(NCORES):
        rows = slice(c * NI, (c + 1) * NI)
        idx = np.arange(c * NI, (c + 1) * NI)

        aux_c = np.empty((NI, N, DE), np.float32)
        np.copyto(aux_c, aux[rows])
        if not all_valid:
            aux_c *= mask_ij[rows][:, :, None]
        for (ci, cj), dv in corr_map.items():
            if c * NI <= ci < (c + 1) * NI:
                aux_c[ci - c * NI, cj, :] += \
                    (m[ci] * m[cj] * dv).astype(np.float32)

        gidx = idx.reshape(NG, IG)                     # (NG, IG) global i

        # lhsT_pre (20, NG*128): ONE pre-matmul per (igroup, half).
        # M-col layout per igroup:
        #   0:64    D features, f-major (f*IG+s), f = a1*4+a2
        #   64:76   ssCaCa tripled (64 + p*IG + s) -> rnorm source
        #   76:96   zero pad (alignment)
        #   96:108  trel (96 + p*IG + s), via the a2=1 rows of rhs_preD:
        #           (-R_i[p]/2)·(-2Ca_j) + (-d_i[p])·1 = R_i[p]·Ca_j - d_i[p]
        #   108:128 zero pad
        lp = np.zeros((20, NG * 128), np.float64)
        Xi = X[gidx]                                   # (NG, IG, 4, 3)
        Xsqi = Xsq[gidx]                               # (NG, IG, 4)
        for f in range(16):
            a1_, a2_ = f // 4, f % 4
            cols = (np.arange(NG)[:, None] * 128 + f * IG + ar4[None, :])
            lp[a2_ * 5 + 0: a2_ * 5 + 3, cols] = \
                Xi[:, :, a1_, :].transpose(2, 0, 1)
            lp[a2_ * 5 + 3, cols] = Xsqi[:, :, a1_]
            lp[a2_ * 5 + 4, cols] = 1.0
        for g in range(NG):
            for s in range(IG):
                i = gidx[g, s]
                for p in range(3):
                    col = g * 128 + 64 + p * IG + s
                    lp[5:8, col] = Ca[i]
                    lp[8, col] = Casq[i]
                    lp[9, col] = 1.0
                    col = g * 128 + 96 + p * IG + s
                    lp[5:8, col] = -R[i, p, :] / 2.0
                    lp[8, col] = -d_i[i, p]

        # scrj (NG, 18, N): rows 0:8 = sc/sep, 8:17 = R_j^T, 17 = ones;
        # DMA'd in one shot into L[76:94]
        ss_rows = np.zeros((NG, 18, N), np.float64)
        ss_rows[:, 8:17, :] = rj_ones[0:9]
        ss_rows[:, 17, :] = 1.0
        for g in range(NG):
            for s in range(IG):
                i = gidx[g, s]
                ss_rows[g, s] = sc_full[i]
                ss_rows[g, IG + s] = sep_full[i]

        # rhs_all (NG, 94, IG*DE)
        ra = np.zeros((NG, KALL, IG * DE), np.float64)
        for g in range(NG):
            for s in range(IG):
                i = gidx[g, s]
                cs = slice(s * DE, (s + 1) * DE)
                mi = m[i]
                for f in range(16):
                    ra[g, f * IG + s, cs] = mi * W_D[f]
                for p in range(3):
                    ra[g, 64 + p * IG + s, cs] = mi * W_T[p]
                ra[g, 76 + s, cs] = mi * w_sc
                ra[g, 80 + s, cs] = mi * w_sep
                ra[g, 84:93, cs] = mi * Wp[i]
                ra[g, 93, cs] = mi * b_edge

        nodeT = node_feat[rows].T
        node_aux = m[rows, None] * (naux[rows].astype(np.float64) + b_node)

        cores.append(dict(
            aux=aux_c,
            rhs_all=np.ascontiguousarray(ra, dtype=np.float32),
            lhsT_preD=lp.astype(np.float32),
            rhs_preD=rhs_preD.astype(np.float32),
            scrj=np.ascontiguousarray(ss_rows, dtype=np.float32),
            maskj=np.ascontiguousarray(m.reshape(6, 128).T),
            nodeT=np.ascontiguousarray(nodeT, dtype=np.float32),
            w_node=W_node.astype(np.float32),
            node_aux=node_aux.astype(np.float32),
            maski=m[rows].reshape(NI, 1).copy(),
        ))

    host_out = dict(
        mask_i=m.reshape(1, N).copy(),
        mask_ij=np.ascontiguousarray(mask_ij.reshape(1, N, N)),
        edge_idx=np.ascontiguousarray(
            np.broadcast_to(np.arange(N, dtype=np.int32)[None, None, :],
                            (1, N, N))),
    )
    return cores, host_out


# ---------------------------------------------------------------------------
# device program
# ---------------------------------------------------------------------------

_NC_CACHE = None
_LAST_RESULTS = None     # stashed BassKernelResults for test harness inspection


def _build_bass(repeat=1):
    import concourse.bass as bass
    import concourse.bacc as bacc
    import concourse.tile as tile
    from concourse import mybir

    AF = mybir.ActivationFunctionType
    ALU = mybir.AluOpType
    f32 = mybir.dt.float32
    f32r = mybir.dt.float32r

    nc = bacc.Bacc("TRN2", debug=False, target_bir_lowering=False)
    d_aux = nc.dram_tensor("aux", (NI, N, DE), f32, kind="ExternalInput")
    d_ra = nc.dram_tensor("rhs_all", (NG, KALL, IG * DE), f32r, kind="ExternalInput")
    d_lp = nc.dram_tensor("lhsT_preD", (20, NG * 128), f32, kind="ExternalInput")
    d_rpD = nc.dram_tensor("rhs_preD", (20, N), f32, kind="ExternalInput")
    d_scrj = nc.dram_tensor("scrj", (NG, 18, N), f32r, kind="ExternalInput")
    d_maskj = nc.dram_tensor("maskj", (128, 6), f32, kind="ExternalInput")
    d_nodeT = nc.dram_tensor("nodeT", (7, NI), f32, kind="ExternalInput")
    d_wn = nc.dram_tensor("w_node", (7, DN), f32, kind="ExternalInput")
    d_naux = nc.dram_tensor("node_aux", (NI, DN), f32, kind="ExternalInput")
    d_maski = nc.dram_tensor("maski", (NI, 1), f32, kind="ExternalInput")
    d_edge = nc.dram_tensor("edge", (NI, N, DE), f32, kind="ExternalOutput")
    d_node = nc.dram_tensor("node", (NI, DN), f32, kind="ExternalOutput")

    with tile.TileContext(nc) as tc, ExitStack() as ctx:
        sp = ctx.enter_context(tc.tile_pool(name="statics", bufs=1))
        lp_ = sp.tile([20, NG * 128], f32)
        rpD_ = sp.tile([20, N], f32)
        maskj_ = sp.tile([128, 6], f32)
        cbias = sp.tile([128, 1], f32)
        nc.gpsimd.memset(cbias[:], 1e-2)
        nc.sync.dma_start(lp_[:], d_lp[:])
        nc.sync.dma_start(rpD_[:], d_rpD[:])
        nc.sync.dma_start(maskj_[:], d_maskj[:])

        lpool = ctx.enter_context(tc.tile_pool(name="L", bufs=4))
        rapool = ctx.enter_context(tc.tile_pool(name="ra", bufs=3))
        ppre = ctx.enter_context(
            tc.tile_pool(name="ppre", bufs=3, space=bass.MemorySpace.PSUM))
        pout = ctx.enter_context(
            tc.tile_pool(name="pout", bufs=4, space=bass.MemorySpace.PSUM))
        pnode = ctx.enter_context(
            tc.tile_pool(name="pnode", bufs=1, space=bass.MemorySpace.PSUM))
        tmp = ctx.enter_context(tc.tile_pool(name="tmp", bufs=4))
        auxp = ctx.enter_context(tc.tile_pool(name="auxp", bufs=4))
        outp = ctx.enter_context(tc.tile_pool(name="outp", bufs=4))

        for _rep in range(repeat):
          for g in range(NG):
            L = lpool.tile([KALL, N], f32r)
            nc.scalar.dma_start(L[76:KALL, :], d_scrj[g])
            ra_ = rapool.tile([KALL, IG * DE], f32r)
            nc.sync.dma_start(ra_[:], d_ra[g])

            for jh in range(2):
                js = slice(jh * 384, (jh + 1) * 384)
                pP = ppre.tile([128, 384], f32)
                nc.tensor.matmul(pP[:], lp_[:, g * 128:(g + 1) * 128],
                                 rpD_[:, js], start=True, stop=True)
                ssc = tmp.tile([76, 384], f32, tag="ssc")
                nc.vector.tensor_scalar(ssc[:], pP[0:76, :], TAU, EPS,
                                        ALU.max, ALU.add)
                nrm = tmp.tile([76, 384], f32, tag="nrm")
                nc.scalar.activation(nrm[:], ssc[:], AF.Sqrt)
                nc.scalar.activation(L[0:64, js], nrm[0:64, :], AF.Ln,
                                     bias=cbias[0:64, :])
                rn = tmp.tile([12, 384], f32, tag="rn")
                nc.vector.reciprocal(rn[:], nrm[64:76, :])
                nc.vector.tensor_tensor(L[64:76, js], pP[96:108, :], rn[:],
                                        ALU.mult)

            at = auxp.tile([128, IG, 6, DE], f32)
            for s in range(IG):
                nc.gpsimd.dma_start(
                    at[:, s, :, :],
                    d_aux[g * IG + s].rearrange("(c p) d -> p c d", p=128))
            ot = outp.tile([128, IG, 6, DE], f32)
            for jc in range(6):
                js = slice(jc * 128, (jc + 1) * 128)
                po = pout.tile([128, IG, DE], f32)
                nc.tensor.matmul(po[:], L[:, js], ra_[:],
                                 start=True, stop=True)
                nc.vector.scalar_tensor_tensor(
                    ot[:, :, jc, :], po[:], maskj_[:, jc:jc + 1],
                    at[:, :, jc, :], ALU.mult, ALU.add)
            for s in range(IG):
                nc.sync.dma_start(
                    d_edge[g * IG + s].rearrange("(c p) d -> p c d", p=128),
                    ot[:, s, :, :])

        # node path
        nt = sp.tile([7, NI], f32)
        wn = sp.tile([7, DN], f32)
        na = sp.tile([NI, DN], f32)
        mi_ = sp.tile([NI, 1], f32)
        nc.sync.dma_start(nt[:], d_nodeT[:])
        nc.sync.dma_start(wn[:], d_wn[:])
        nc.sync.dma_start(na[:], d_naux[:])
        nc.sync.dma_start(mi_[:], d_maski[:])
        pn = pnode.tile([NI, DN], f32)
        nc.tensor.matmul(pn[:], nt[:], wn[:], start=True, stop=True)
        no = sp.tile([NI, DN], f32, tag="nodeout")
        nc.vector.scalar_tensor_tensor(no[:], pn[:], mi_[:], na[:],
                                       ALU.mult, ALU.add)
        nc.sync.dma_start(d_node[:], no[:])

    nc.compile()
    return nc


def _get_nc():
    global _NC_CACHE
    if _NC_CACHE is None:
        _NC_CACHE = _build_bass()
    return _NC_CACHE


def kernel(X, C, node_h_aux, edge_h_aux, W_node, b_node, W_edge, b_edge):
    global _LAST_RESULTS
    from concourse import bass_utils

    cores, host_out = _prep(X, C, node_h_aux, edge_h_aux,
                            W_node, b_node, W_edge, b_edge)
    nc = _get_nc()
    res = bass_utils.run_bass_kernel_spmd(
        nc, cores, core_ids=list(range(NCORES)))
    _LAST_RESULTS = res

    node_h = np.concatenate(
        [res.results[c]["node"] for c in range(NCORES)], 0)[None]
    edge_h = np.concatenate(
        [res.results[c]["edge"] for c in range(NCORES)], 0)[None]
    return (node_h.astype(np.float32),
            edge_h.astype(np.float32),
            host_out["edge_idx"],
            host_out["mask_i"].astype(np.float32),
            host_out["mask_ij"].astype(np.float32))
